# revision 13
# baseline (speedup 1.0000x reference)
"""Trainium2 Bass kernel for the batched ConstituencyTreeLSTM cell.

Data-parallel across 8 NeuronCores: each core processes 12500 nodes
(padded to 12544 = 98 micro-tiles of 128 nodes, 7 per group).

fp8 (e4m3, TRN ±240) DoubleRow matmuls carry the large input chunks:
  leaf:  [e;h_prev] as two DR pairs (4 planes x 128 rows, K_eff=256) vs
         fp8 weights; tags+bias chunk stays bf16.  PSUM [128, i|o|fl|u],
         all four gates evaluated with ONE sigmoid per micro-tile
         (u-gate weights pre-scaled x2 so tanh(u) = 2*sig(2u)-1).
  node:  tags chunk + Uh(h1^T rows 0:128, via DMA xbar transpose) in
         bf16; DR pair (k[0:128] | [h1^T tail rows; k tail rows]) in fp8
         (h1 tail transposed on PE, cast to fp8 by the DVE evac copy).
         PSUM [128, i2|o2|fl2|fd2|u2], one sigmoid per micro-tile.
All matmul weights are pre-scaled x64 (fp8 range use / denormal
avoidance); the sigmoid activation applies scale=1/64 for free.
tanh(c1)/tanh(c2) run per-group on ACT; gate algebra on DVE with the
2s-1 affine on GPSIMD.  Pipeline: node block of group g runs two
iterations after its leaf block (keeps PE HAM warm through loads).
"""

import os
import sys

import numpy as np

try:
    import concourse.bass as bass  # noqa: F401
except Exception:  # pragma: no cover - fallback for bare environments
    for p in (
        "/root/.axon_site",
        "/root/.axon_site/_ro/trn_rl_repo",
        "/root/.axon_site/_ro/pypackages",
        "/opt/trn_rl_repo",
        "/opt/pypackages",
    ):
        if os.path.isdir(p) and p not in sys.path:
            sys.path.append(p)
    import concourse.bass as bass  # noqa: F401

import ml_dtypes
import concourse.mybir as mybir
import concourse.tile as tile
from concourse import bacc
from concourse.bass_utils import run_bass_kernel_spmd
from concourse.masks import make_identity

BF16 = ml_dtypes.bfloat16
F8 = ml_dtypes.float8_e4m3  # TRN fp8e4: max +-240, S.1111.000 = inf

N_CORES = 8
N = 100000
NPER = N // N_CORES            # 12500
MICRO = 128                    # nodes per matmul tile
GRP = 7                        # micro tiles per group
GNODES = MICRO * GRP           # 896
NGRP = 14                      # groups per core
NPAD = NGRP * GNODES           # 12544
M = 168                        # mem dim

WS = 64.0                      # fp8 weight pre-scale (1/WS folded into sig)

F32 = mybir.dt.float32
BF = mybir.dt.bfloat16
FP8 = mybir.dt.float8e4
SIGF = mybir.ActivationFunctionType.Sigmoid
TANHF = mybir.ActivationFunctionType.Tanh
DR = mybir.MatmulPerfMode.DoubleRow
MULT = mybir.AluOpType.mult
ADD = mybir.AluOpType.add

# fp8 weight pack column layout (1B/col): leaf DR pairs (2 planes x 336)
# and the node k-pair (2 planes x 420, plane stride padded to 432).
W8_L1A, W8_L1B = 0, 672
W8_L2A, W8_L2B = 1344, 2016
W8_KA, W8_KB = 2688, 3552
W8_COLS = 4416
# bf16 weight pack: leaf tags (2x336), node tags (2x420), node Uh (2x420)
WB_TLA, WB_TLB = 0, 336
WB_TNA, WB_TNB = 672, 1092
WB_UHA, WB_UHB = 1512, 1932
WB_COLS = 2352

_compiled = None
LAST_RESULT = None


def _build(ngrp=NGRP):
    npad = ngrp * GNODES
    nc = bacc.Bacc("TRN2", target_bir_lowering=False, debug=False,
                   num_devices=N_CORES)

    x8_d = nc.dram_tensor("x8", [512, npad], FP8, kind="ExternalInput")
    k8_d = nc.dram_tensor("k8", [256, npad], FP8, kind="ExternalInput")
    tt1T_d = nc.dram_tensor("tt1T", [101, npad], BF, kind="ExternalInput")
    cq_d = nc.dram_tensor("cq", [npad, 336], BF, kind="ExternalInput")
    w8_d = nc.dram_tensor("w8", [128, W8_COLS], FP8, kind="ExternalInput")
    wb_d = nc.dram_tensor("wb", [128, WB_COLS], BF, kind="ExternalInput")
    out_d = nc.dram_tensor("out", [npad, 336], BF, kind="ExternalOutput")

    with tile.TileContext(nc) as tc:
        from contextlib import ExitStack
        with ExitStack() as ctx:
            wpool = ctx.enter_context(tc.tile_pool(name="w", bufs=1))
            spool = ctx.enter_context(tc.tile_pool(name="s", bufs=5))
            opool = ctx.enter_context(tc.tile_pool(name="o", bufs=2))
            gpool = ctx.enter_context(tc.tile_pool(name="g", bufs=2))
            c1pool = ctx.enter_context(tc.tile_pool(name="c1p", bufs=3))
            tpool = ctx.enter_context(tc.tile_pool(name="tp", bufs=2))
            fpool = ctx.enter_context(
                tc.tile_pool(name="fp", bufs=2, space="PSUM"))

            # ---- constants / weights (resident) ----
            ident = wpool.tile([128, 128], BF, tag="ident")
            make_identity(nc, ident[:])
            w8 = wpool.tile([128, W8_COLS], FP8, tag="w8")
            nc.sync.dma_start(w8[:], w8_d[:, :])
            wb = wpool.tile([128, WB_COLS], BF, tag="wb")
            nc.sync.dma_start(wb[:], wb_d[:, :])

            def w8r(base, n, stride):
                blk = w8[:, base:base + 2 * stride]
                return blk.rearrange("p (a b) -> p a b", a=2,
                                     b=stride)[:, :, 0:n]

            sstate = {}
            gstate = {}

            def load_group_main(g, split=False):
                cs = g * GNODES
                EK = spool.tile([128, 4, GNODES], FP8, tag="EK")
                K8 = spool.tile([128, 2, GNODES], FP8, tag="K8")
                TTs = spool.tile([128, GNODES], BF, tag="TT")
                h = GNODES // 2
                rngs = ((0, h), (h, GNODES)) if split else ((0, GNODES),)
                # startup groups: spread the load burst across both rings
                eng2 = nc.gpsimd if split else nc.sync
                for c0, c1 in rngs:
                    nc.sync.dma_start(
                        EK[:, :, c0:c1],
                        x8_d[:, cs + c0:cs + c1].rearrange(
                            "(c p) n -> p c n", p=128))
                    eng2.dma_start(
                        K8[:, :, c0:c1],
                        k8_d[:, cs + c0:cs + c1].rearrange(
                            "(c p) n -> p c n", p=128))
                    eng2.dma_start(TTs[0:101, c0:c1],
                                   tt1T_d[:, cs + c0:cs + c1])
                Bt = spool.tile([128, GNODES], BF, tag="B")
                sstate[g] = dict(EK=EK, K8=K8, TT=TTs, B=Bt)

            def load_group_cq(g):
                cs = g * GNODES
                CQt = spool.tile([128, GRP, 336], BF, tag="CQ")
                nc.gpsimd.dma_start(
                    CQt[:],
                    cq_d[cs:cs + GNODES, :].rearrange("(m p) f -> p m f",
                                                      p=128))
                sstate[g].update(CQt=CQt)

            def fused_block(gl, gn):
                """Interleaved leaf(gl) + node(gn) matmuls, one merged
                sigmoid ACT per micro-tile over the 4-bank fused PSUM tile.
                sga flat layout per micro: [i,o,x84,fl,u,x84 | i2,o2,fl2,fd2,u2]
                (x84 = garbage from the rectangular [., 420] ACT window)."""
                sga = gpool.tile([128, GRP, 1680], BF, tag="sga")
                stl = sstate[gl] if gl is not None else None
                stn = sstate[gn] if gn is not None else None
                for j in range(GRP):
                    c0 = j * MICRO
                    P = fpool.tile([128, 2048], F32, tag="P")
                    if gl is not None:
                        EK, TTs = stl["EK"], stl["TT"]
                        XA = EK[:, 0:2, c0:c0 + MICRO]
                        XB = EK[:, 2:4, c0:c0 + MICRO]
                        XT = TTs[0:101, c0:c0 + MICRO]
                        nc.tensor.matmul(P[:, 0:336], XA, w8r(W8_L1A, 336, 336),
                                         start=True, stop=False, perf_mode=DR)
                        nc.tensor.matmul(P[:, 512:848], XA, w8r(W8_L1B, 336, 336),
                                         start=True, stop=False, perf_mode=DR)
                        nc.tensor.matmul(P[:, 0:336], XB, w8r(W8_L2A, 336, 336),
                                         start=False, stop=False, perf_mode=DR)
                        nc.tensor.matmul(P[:, 512:848], XB, w8r(W8_L2B, 336, 336),
                                         start=False, stop=False, perf_mode=DR)
                        nc.tensor.matmul(P[:, 0:336], XT,
                                         wb[0:101, WB_TLA:WB_TLA + 336],
                                         start=False, stop=True)
                        nc.tensor.matmul(P[:, 512:848], XT,
                                         wb[0:101, WB_TLB:WB_TLB + 336],
                                         start=False, stop=True)
                    if gn is not None:
                        TTn, Btn, K8n = stn["TT"], stn["B"], stn["K8"]
                        XTn = TTn[0:101, c0:c0 + MICRO]
                        XBn = Btn[:, c0:c0 + MICRO]
                        XKn = K8n[:, 0:2, c0:c0 + MICRO]
                        nc.tensor.matmul(P[:, 1024:1444], XTn,
                                         wb[0:101, WB_TNA:WB_TNA + 420],
                                         start=True, stop=False)
                        nc.tensor.matmul(P[:, 1536:1956], XTn,
                                         wb[0:101, WB_TNB:WB_TNB + 420],
                                         start=True, stop=False)
                        nc.tensor.matmul(P[:, 1024:1444], XBn,
                                         wb[:, WB_UHA:WB_UHA + 420],
                                         start=False, stop=False)
                        nc.tensor.matmul(P[:, 1536:1956], XBn,
                                         wb[:, WB_UHB:WB_UHB + 420],
                                         start=False, stop=False)
                        nc.tensor.matmul(P[:, 1024:1444], XKn,
                                         w8r(W8_KA, 420, 432),
                                         start=False, stop=True, perf_mode=DR)
                        nc.tensor.matmul(P[:, 1536:1956], XKn,
                                         w8r(W8_KB, 420, 432),
                                         start=False, stop=True, perf_mode=DR)
                    pr = P[:].rearrange("p (a b) -> p a b", a=4, b=512)
                    sr = sga[:, j, :].rearrange("p (a b) -> p a b", a=4, b=420)
                    if gl is not None and gn is not None:
                        nc.scalar.activation(sr, pr[:, :, 0:420], SIGF,
                                             scale=1.0 / WS)
                    elif gl is not None:
                        nc.scalar.activation(sr[:, 0:2], pr[:, 0:2, 0:420],
                                             SIGF, scale=1.0 / WS)
                    else:
                        nc.scalar.activation(sr[:, 2:4], pr[:, 2:4, 0:420],
                                             SIGF, scale=1.0 / WS)
                if gl is not None:
                    gstate[gl] = dict()
                return sga

            def chain_pre(g, sga):
                """Gate algebra for c1(g) (DVE), issued at iteration end."""
                st = sstate[g]
                gs = gstate[g]
                CQt = st["CQt"]
                s_i = sga[:, :, 0:168]
                s_fl = sga[:, :, 420:588]
                s_u = sga[:, :, 588:756]
                tu = gpool.tile([128, GRP, 168], BF, tag="tu")
                nc.gpsimd.tensor_scalar(tu[:], s_u, 2.0, -1.0, MULT, ADD)
                m1 = gpool.tile([128, GRP, 168], BF, tag="tmpA")
                nc.vector.tensor_mul(m1[:], s_i, tu[:])
                m2 = gpool.tile([128, GRP, 168], BF, tag="tmpB")
                nc.vector.tensor_mul(m2[:], s_fl, CQt[:, :, 0:168])
                c1t = c1pool.tile([128, GRP, 168], BF, tag="c1")
                nc.vector.tensor_add(c1t[:], m1[:], m2[:])
                gs["c1"] = c1t
                gs["sga"] = sga

            def chain_post(g):
                """tanh(c1), h1, and the h1 transposes for group g — issued
                at the START of iteration g+1 so the tanh leads the scalar
                queue and Bt/K8 are ready a full iteration before node(g)."""
                st = sstate[g]
                gs = gstate[g]
                Bt, K8 = st["B"], st["K8"]
                sga = gs["sga"]
                c1t = gs["c1"]
                s_o = sga[:, :, 168:336]
                tc1 = gpool.tile([128, GRP, 168], BF, tag="tc1")
                nc.scalar.activation(tc1[:], c1t[:], TANHF)
                # h1 split into xbar-transposable contiguous pieces:
                # main rows 0:128 and tail rows 128:168 (tail padded to a
                # 128-col tile; cols 40:128 are never-read garbage).
                h1m = c1pool.tile([128, GRP, 128], BF, tag="h1m")
                nc.vector.tensor_mul(h1m[:], s_o[:, :, 0:128], tc1[:, :, 0:128])
                h1tl = tpool.tile([128, GRP, 128], BF, tag="h1tl")
                nc.vector.tensor_mul(h1tl[:, :, 0:40], s_o[:, :, 128:168],
                                     tc1[:, :, 128:168])
                # one batched 7x(128x128) xbar transpose per group for the
                # main rows, one for the padded tail (both sync ring), then
                # one grouped cast tail -> fp8 K8 plane 1 (gpsimd so its
                # DMA-completion wait does not head-block the vector queue).
                nc.sync.dma_start_transpose(
                    Bt[:].rearrange("p (j c) -> p j c", j=GRP), h1m[:])
                tT = tpool.tile([128, GRP, 128], BF, tag="tT")
                nc.sync.dma_start_transpose(tT[:], h1tl[:])
                nc.gpsimd.tensor_copy(K8[0:40, 1, :],
                                      tT[0:40].rearrange("p j c -> p (j c)"))

            def node_algebra(g, sga):
                st = sstate[g]
                CQt = st["CQt"]
                c1t = gstate[g]["c1"]
                OUTt = opool.tile([128, GRP, 336], BF, tag="OUT")
                s_i2 = sga[:, :, 840:1008]
                s_o2 = sga[:, :, 1008:1176]
                s_fl2 = sga[:, :, 1176:1344]
                s_fd2 = sga[:, :, 1344:1512]
                s_u2 = sga[:, :, 1512:1680]
                tu2 = gpool.tile([128, GRP, 168], BF, tag="tu2")
                nc.gpsimd.tensor_scalar(tu2[:], s_u2, 2.0, -1.0, MULT, ADD)
                m3 = gpool.tile([128, GRP, 168], BF, tag="tmpA")
                nc.vector.tensor_mul(m3[:], s_i2, tu2[:])
                m4 = gpool.tile([128, GRP, 168], BF, tag="tmpB")
                nc.vector.tensor_mul(m4[:], s_fd2, CQt[:, :, 168:336])
                m5 = gpool.tile([128, GRP, 168], BF, tag="tmpC")
                nc.vector.tensor_mul(m5[:], s_fl2, c1t[:])
                a1 = gpool.tile([128, GRP, 168], BF, tag="tmpA")
                nc.vector.tensor_add(a1[:], m3[:], m4[:])
                nc.vector.tensor_add(OUTt[:, :, 168:336], a1[:], m5[:])
                tc2 = gpool.tile([128, GRP, 168], BF, tag="tc2")
                nc.scalar.activation(tc2[:], OUTt[:, :, 168:336], TANHF)
                nc.vector.tensor_mul(OUTt[:, :, 0:168], s_o2, tc2[:])
                return OUTt

            def store_group(g, OUTt):
                cs = g * GNODES
                nc.gpsimd.dma_start(
                    out_d[cs:cs + GNODES, :].rearrange("(m p) f -> p m f",
                                                       p=128),
                    OUTt[:])

            load_group_main(0, split=True)
            if ngrp > 1:
                load_group_main(1, split=True)
            # zero the leaf-bank garbage windows (cols 336:420 per leaf
            # bank) of both fused PSUM buffers so the rectangular merged
            # ACT never reads uninitialized PSUM.
            warm = None
            for _ in range(2):
                Pz = fpool.tile([128, 2048], F32, tag="P")
                nc.vector.memset(Pz[:, 336:420], 0.0)
                nc.vector.memset(Pz[:, 848:932], 0.0)
                # bank-3 tail, never used by real matmuls: HAM warm target
                warm = Pz[:, 1980:2044]
            # warm up the PE HAM clock gate with dummy matmuls while the
            # first input tiles stream in (cold PE runs at half clock).
            for _ in range(110):
                nc.tensor.matmul(warm, ident[:], ident[:, 0:64],
                                 start=True, stop=True)
            for g in range(ngrp + 2):
                gl = g if g < ngrp else None
                gn = g - 2 if g >= 2 else None
                if 1 <= g <= ngrp:
                    chain_post(g - 1)
                if g == 0:
                    load_group_cq(0)
                sga = fused_block(gl, gn)
                if gl is not None and gl < 2:
                    # keep the PE HAM warm through the startup load waits;
                    # reading this group's sigmoid output pins these after
                    # the leaf block (else the scheduler hoists them early)
                    for _ in range(120 if g == 0 else 60):
                        nc.tensor.matmul(warm, ident[:], sga[:, 0, 0:64],
                                         start=True, stop=True)
                if gn is not None:
                    store_group(gn, node_algebra(gn, sga))
                if gl is not None and gl + 2 < ngrp:
                    load_group_main(gl + 2)
                if gl is not None and gl + 1 < ngrp:
                    load_group_cq(gl + 1)
                if gl is not None:
                    chain_pre(gl, sga)

    nc.compile()
    return nc


def _q8(a):
    return np.clip(np.asarray(a, np.float32), -240, 240).astype(F8)


def _prep_core(inputs, c, npad=NPAD, nper=NPER):
    """Build the per-core (sharded, transposed, fp8/bf16) input arrays."""
    sl = slice(c * nper, (c + 1) * nper)
    e = inputs["e"][sl]
    h_prev = inputs["h_prev"][sl]
    tag = inputs["tag"][sl]
    tagp = inputs["tag_parent"][sl]
    k = inputs["k"][sl]
    c_prev = inputs["c_prev"][sl]
    q = inputs["q"][sl]
    n = e.shape[0]

    x8 = np.zeros((512, npad), F8)
    x8[0:300, :n] = _q8(e.T)
    x8[300:468, :n] = _q8(h_prev.T)
    k8 = np.zeros((256, npad), F8)
    k8[0:128, :n] = _q8(k[:, 0:128].T)
    # plane 1: partitions 0:40 are the device-written h1 tail slot;
    # 40:80 carry k tail rows; rest zero.
    k8[168:208, :n] = _q8(k[:, 128:168].T)
    tt1T = np.zeros((101, npad), BF16)
    tt1T[0:50, :n] = tag.T
    tt1T[50:100, :n] = tagp.T
    tt1T[100, :n] = 1.0
    cq = np.zeros((npad, 336), BF16)
    cq[:n, 0:168] = c_prev
    cq[:n, 168:336] = q
    return dict(x8=x8, k8=k8, tt1T=tt1T, cq=cq)


def _prep_weights(inputs):
    cat = np.concatenate
    # leaf fused weights [569, 672]: rows [e(300); h_prev(168); tags(101)],
    # gate cols [i,o,fl,u]; u columns pre-scaled x2 (tanh via sigmoid),
    # everything x WS for fp8 range (sigmoid applies 1/WS).
    w_l = cat([inputs["We_l"], inputs["Uh_l"],
               inputs["Wt_l"], inputs["Wtp_l"],
               inputs["b_l"][None, :]], 0).astype(np.float64) * WS
    w_l[:, 504:672] *= 2.0
    # node fused [438, 840]: rows [tags(101); h1(168); k(168)],
    # gate cols [i,o,fl,fd,u] (u cols x2); Uk covers gates 0:672 only.
    w_n = np.zeros((437, 840), np.float64)
    w_n[0:50] = inputs["Wt_n"]
    w_n[50:100] = inputs["Wtp_n"]
    w_n[100] = inputs["b_n"]
    w_n[101:269] = inputs["Uh_n"]
    w_n[269:437, 0:672] = inputs["Uk_n"]
    w_n *= WS
    w_n[:, 672:840] *= 2.0

    w8 = np.zeros((128, W8_COLS), F8)

    def put8(base, stride, n, pl0, pl1):
        # pl0/pl1: [rows<=128, n] weight blocks for the two DR planes
        for i, pl in enumerate((pl0, pl1)):
            w8[0:pl.shape[0], base + i * stride:base + i * stride + n] = \
                _q8(pl)

    # leaf pair 1: planes e[0:128], e[128:256]
    put8(W8_L1A, 336, 336, w_l[0:128, 0:336], w_l[128:256, 0:336])
    put8(W8_L1B, 336, 336, w_l[0:128, 336:672], w_l[128:256, 336:672])
    # leaf pair 2: planes [e256:300;h0:84], [h84:168; pad44]
    put8(W8_L2A, 336, 336, w_l[256:384, 0:336], w_l[384:468, 0:336])
    put8(W8_L2B, 336, 336, w_l[256:384, 336:672], w_l[384:468, 336:672])
    # node k pair: plane0 = k rows 0:128 (Uk), plane1 = [h1 tail rows
    # 128:168 (Uh) at 0:40; k tail rows at 40:80]
    kp0 = w_n[269:397]
    kp1 = np.zeros((80, 840))
    kp1[0:40] = w_n[229:269]   # Uh rows 128:168
    kp1[40:80] = w_n[397:437]  # Uk rows 128:168
    put8(W8_KA, 432, 420, kp0[:, 0:420], kp1[:, 0:420])
    put8(W8_KB, 432, 420, kp0[:, 420:840], kp1[:, 420:840])

    wbp = np.zeros((128, WB_COLS), BF16)
    tl = w_l[468:569]          # leaf tags rows [101, 672]
    wbp[0:101, WB_TLA:WB_TLA + 336] = tl[:, 0:336].astype(BF16)
    wbp[0:101, WB_TLB:WB_TLB + 336] = tl[:, 336:672].astype(BF16)
    tn = w_n[0:101]            # node tags rows [101, 840]
    wbp[0:101, WB_TNA:WB_TNA + 420] = tn[:, 0:420].astype(BF16)
    wbp[0:101, WB_TNB:WB_TNB + 420] = tn[:, 420:840].astype(BF16)
    uh = w_n[101:229]          # Uh rows 0:128 [128, 840]
    wbp[:, WB_UHA:WB_UHA + 420] = uh[:, 0:420].astype(BF16)
    wbp[:, WB_UHB:WB_UHB + 420] = uh[:, 420:840].astype(BF16)
    return dict(w8=w8, wb=wbp)


def kernel(**inputs):
    global _compiled, LAST_RESULT
    inputs = {k: np.asarray(v) for k, v in inputs.items()}
    if _compiled is None:
        _compiled = _build()
    weights = _prep_weights(inputs)
    in_maps = []
    for c in range(N_CORES):
        m = _prep_core(inputs, c)
        m.update(weights)
        in_maps.append(m)
    res = run_bass_kernel_spmd(_compiled, in_maps,
                               core_ids=list(range(N_CORES)))
    LAST_RESULT = res
    outs = [res.results[c]["out"][:NPER].astype(np.float32)
            for c in range(N_CORES)]
    return np.concatenate(outs, 0)



# revision 18
# speedup vs baseline: 1.0775x; 1.0775x over previous
"""Trainium2 Bass kernel for the batched ConstituencyTreeLSTM cell.

Data-parallel across 8 NeuronCores: each core processes 12500 nodes
(padded to 12544 = 98 micro-tiles of 128 nodes, 7 per group).

fp8 (e4m3, TRN ±240) DoubleRow matmuls carry the large input chunks:
  leaf:  [e;h_prev] as two DR pairs (4 planes x 128 rows, K_eff=256) vs
         fp8 weights; tags+bias chunk stays bf16.  PSUM [128, i|o|fl|u],
         all four gates evaluated with ONE sigmoid per micro-tile
         (u-gate weights pre-scaled x2 so tanh(u) = 2*sig(2u)-1).
  node:  tags chunk + Uh(h1^T rows 0:128, via DMA xbar transpose) in
         bf16; DR pair (k[0:128] | [h1^T tail rows; k tail rows]) in fp8
         (h1 tail transposed on PE, cast to fp8 by the DVE evac copy).
         PSUM [128, i2|o2|fl2|fd2|u2], one sigmoid per micro-tile.
All matmul weights are pre-scaled x64 (fp8 range use / denormal
avoidance); the sigmoid activation applies scale=1/64 for free.
tanh(c1)/tanh(c2) run per-group on ACT; gate algebra on DVE with the
2s-1 affine on GPSIMD.  Pipeline: node block of group g runs two
iterations after its leaf block (keeps PE HAM warm through loads).
"""

import os
import sys

import numpy as np

try:
    import concourse.bass as bass  # noqa: F401
except Exception:  # pragma: no cover - fallback for bare environments
    for p in (
        "/root/.axon_site",
        "/root/.axon_site/_ro/trn_rl_repo",
        "/root/.axon_site/_ro/pypackages",
        "/opt/trn_rl_repo",
        "/opt/pypackages",
    ):
        if os.path.isdir(p) and p not in sys.path:
            sys.path.append(p)
    import concourse.bass as bass  # noqa: F401

import ml_dtypes
import concourse.mybir as mybir
import concourse.tile as tile
from concourse import bacc
from concourse.bass_utils import run_bass_kernel_spmd
from concourse.masks import make_identity

BF16 = ml_dtypes.bfloat16
F8 = ml_dtypes.float8_e4m3  # TRN fp8e4: max +-240, S.1111.000 = inf

N_CORES = 8
N = 100000
NPER = N // N_CORES            # 12500
MICRO = 128                    # nodes per matmul tile
GRP = 7                        # micro tiles per group
GNODES = MICRO * GRP           # 896
NGRP = 14                      # groups per core
NPAD = NGRP * GNODES           # 12544
M = 168                        # mem dim
NL = 3                         # leaf->node pipeline lag (groups)

WS = 64.0                      # fp8 weight pre-scale (1/WS folded into sig)

F32 = mybir.dt.float32
BF = mybir.dt.bfloat16
FP8 = mybir.dt.float8e4
SIGF = mybir.ActivationFunctionType.Sigmoid
TANHF = mybir.ActivationFunctionType.Tanh
DR = mybir.MatmulPerfMode.DoubleRow
MULT = mybir.AluOpType.mult
ADD = mybir.AluOpType.add

# fp8 weight pack column layout (1B/col): leaf DR pairs (2 planes x 336)
# and the node k-pair (2 planes x 420, plane stride padded to 432).
W8_L1A, W8_L1B = 0, 672
W8_L2A, W8_L2B = 1344, 2016
W8_KA, W8_KB = 2688, 3552
W8_COLS = 4416
# bf16 weight pack: leaf tags (2x336), node tags (2x420), node Uh (2x420)
WB_TLA, WB_TLB = 0, 336
WB_TNA, WB_TNB = 672, 1092
WB_UHA, WB_UHB = 1512, 1932
WB_COLS = 2352

_compiled = None
LAST_RESULT = None


def _build(ngrp=NGRP):
    npad = ngrp * GNODES
    nc = bacc.Bacc("TRN2", target_bir_lowering=False, debug=False,
                   num_devices=N_CORES)

    x8_d = nc.dram_tensor("x8", [512, npad], FP8, kind="ExternalInput")
    k8_d = nc.dram_tensor("k8", [256, npad], FP8, kind="ExternalInput")
    tt1T_d = nc.dram_tensor("tt1T", [101, npad], BF, kind="ExternalInput")
    cq_d = nc.dram_tensor("cq", [npad, 336], BF, kind="ExternalInput")
    w8_d = nc.dram_tensor("w8", [128, W8_COLS], FP8, kind="ExternalInput")
    wb_d = nc.dram_tensor("wb", [128, WB_COLS], BF, kind="ExternalInput")
    out_d = nc.dram_tensor("out", [npad, 336], BF, kind="ExternalOutput")

    with tile.TileContext(nc) as tc:
        from contextlib import ExitStack
        with ExitStack() as ctx:
            wpool = ctx.enter_context(tc.tile_pool(name="w", bufs=1))
            epool = ctx.enter_context(tc.tile_pool(name="e", bufs=3))
            npool = ctx.enter_context(tc.tile_pool(name="n", bufs=NL + 3))
            opool = ctx.enter_context(tc.tile_pool(name="o", bufs=2))
            gpool = ctx.enter_context(tc.tile_pool(name="g", bufs=2))
            c1pool = ctx.enter_context(tc.tile_pool(name="c1p", bufs=NL + 1))
            tpool = ctx.enter_context(tc.tile_pool(name="tp", bufs=2))
            fpool = ctx.enter_context(
                tc.tile_pool(name="fp", bufs=2, space="PSUM"))

            # ---- constants / weights (resident) ----
            ident = wpool.tile([128, 128], BF, tag="ident")
            make_identity(nc, ident[:])
            w8 = wpool.tile([128, W8_COLS], FP8, tag="w8")
            nc.sync.dma_start(w8[:], w8_d[:, :])
            wb = wpool.tile([128, WB_COLS], BF, tag="wb")
            nc.sync.dma_start(wb[:], wb_d[:, :])

            def w8r(base, n, stride):
                blk = w8[:, base:base + 2 * stride]
                return blk.rearrange("p (a b) -> p a b", a=2,
                                     b=stride)[:, :, 0:n]

            sstate = {}
            gstate = {}

            def load_group_main(g, split=False):
                cs = g * GNODES
                EK = epool.tile([128, 4, GNODES], FP8, tag="EK")
                K8 = npool.tile([128, 2, GNODES], FP8, tag="K8")
                TTs = npool.tile([128, GNODES], BF, tag="TT")
                h = GNODES // 2
                rngs = ((0, h), (h, GNODES)) if split else ((0, GNODES),)
                # startup groups: spread the load burst across both rings
                eng2 = nc.gpsimd if split else nc.sync
                for c0, c1 in rngs:
                    nc.sync.dma_start(
                        EK[:, :, c0:c1],
                        x8_d[:, cs + c0:cs + c1].rearrange(
                            "(c p) n -> p c n", p=128))
                    eng2.dma_start(
                        K8[:, :, c0:c1],
                        k8_d[:, cs + c0:cs + c1].rearrange(
                            "(c p) n -> p c n", p=128))
                    eng2.dma_start(TTs[0:101, c0:c1],
                                   tt1T_d[:, cs + c0:cs + c1])
                Bt = npool.tile([128, GNODES], BF, tag="B")
                sstate[g] = dict(EK=EK, K8=K8, TT=TTs, B=Bt)

            def load_group_cq(g):
                cs = g * GNODES
                CQt = npool.tile([128, GRP, 336], BF, tag="CQ")
                nc.gpsimd.dma_start(
                    CQt[:],
                    cq_d[cs:cs + GNODES, :].rearrange("(m p) f -> p m f",
                                                      p=128))
                sstate[g].update(CQt=CQt)

            def fused_block(gl, gn):
                """Interleaved leaf(gl) + node(gn) matmuls, one merged
                sigmoid ACT per micro-tile over the 4-bank fused PSUM tile.
                sga flat layout per micro: [i,o,x84,fl,u,x84 | i2,o2,fl2,fd2,u2]
                (x84 = garbage from the rectangular [., 420] ACT window)."""
                sga = gpool.tile([128, GRP, 1680], BF, tag="sga")
                stl = sstate[gl] if gl is not None else None
                stn = sstate[gn] if gn is not None else None
                for j in range(GRP):
                    c0 = j * MICRO
                    P = fpool.tile([128, 2048], F32, tag="P")
                    if gl is not None:
                        EK, TTs = stl["EK"], stl["TT"]
                        XA = EK[:, 0:2, c0:c0 + MICRO]
                        XB = EK[:, 2:4, c0:c0 + MICRO]
                        XT = TTs[0:101, c0:c0 + MICRO]
                        nc.tensor.matmul(P[:, 0:336], XA, w8r(W8_L1A, 336, 336),
                                         start=True, stop=False, perf_mode=DR)
                        nc.tensor.matmul(P[:, 512:848], XA, w8r(W8_L1B, 336, 336),
                                         start=True, stop=False, perf_mode=DR)
                        nc.tensor.matmul(P[:, 0:336], XB, w8r(W8_L2A, 336, 336),
                                         start=False, stop=False, perf_mode=DR)
                        nc.tensor.matmul(P[:, 512:848], XB, w8r(W8_L2B, 336, 336),
                                         start=False, stop=False, perf_mode=DR)
                        nc.tensor.matmul(P[:, 0:336], XT,
                                         wb[0:101, WB_TLA:WB_TLA + 336],
                                         start=False, stop=True)
                        nc.tensor.matmul(P[:, 512:848], XT,
                                         wb[0:101, WB_TLB:WB_TLB + 336],
                                         start=False, stop=True)
                    if gn is not None:
                        TTn, Btn, K8n = stn["TT"], stn["B"], stn["K8"]
                        XTn = TTn[0:101, c0:c0 + MICRO]
                        XBn = Btn[:, c0:c0 + MICRO]
                        XKn = K8n[:, 0:2, c0:c0 + MICRO]
                        nc.tensor.matmul(P[:, 1024:1444], XTn,
                                         wb[0:101, WB_TNA:WB_TNA + 420],
                                         start=True, stop=False)
                        nc.tensor.matmul(P[:, 1536:1956], XTn,
                                         wb[0:101, WB_TNB:WB_TNB + 420],
                                         start=True, stop=False)
                        nc.tensor.matmul(P[:, 1024:1444], XBn,
                                         wb[:, WB_UHA:WB_UHA + 420],
                                         start=False, stop=False)
                        nc.tensor.matmul(P[:, 1536:1956], XBn,
                                         wb[:, WB_UHB:WB_UHB + 420],
                                         start=False, stop=False)
                        nc.tensor.matmul(P[:, 1024:1444], XKn,
                                         w8r(W8_KA, 420, 432),
                                         start=False, stop=True, perf_mode=DR)
                        nc.tensor.matmul(P[:, 1536:1956], XKn,
                                         w8r(W8_KB, 420, 432),
                                         start=False, stop=True, perf_mode=DR)
                    pr = P[:].rearrange("p (a b) -> p a b", a=4, b=512)
                    sr = sga[:, j, :].rearrange("p (a b) -> p a b", a=4, b=420)
                    if gl is not None and gn is not None:
                        nc.scalar.activation(sr, pr[:, :, 0:420], SIGF,
                                             scale=1.0 / WS)
                    elif gl is not None:
                        nc.scalar.activation(sr[:, 0:2], pr[:, 0:2, 0:420],
                                             SIGF, scale=1.0 / WS)
                    else:
                        nc.scalar.activation(sr[:, 2:4], pr[:, 2:4, 0:420],
                                             SIGF, scale=1.0 / WS)
                if gl is not None:
                    gstate[gl] = dict()
                return sga

            def chain_block(g, sga):
                st = sstate[g]
                gs = gstate[g]
                CQt, Bt, K8 = st["CQt"], st["B"], st["K8"]
                s_i = sga[:, :, 0:168]
                s_o = sga[:, :, 168:336]
                s_fl = sga[:, :, 420:588]
                s_u = sga[:, :, 588:756]
                tu = gpool.tile([128, GRP, 168], BF, tag="tu")
                nc.gpsimd.tensor_scalar(tu[:], s_u, 2.0, -1.0, MULT, ADD)
                m1 = gpool.tile([128, GRP, 168], BF, tag="tmpA")
                nc.vector.tensor_mul(m1[:], s_i, tu[:])
                m2 = gpool.tile([128, GRP, 168], BF, tag="tmpB")
                nc.vector.tensor_mul(m2[:], s_fl, CQt[:, :, 0:168])
                c1t = c1pool.tile([128, GRP, 168], BF, tag="c1")
                nc.vector.tensor_add(c1t[:], m1[:], m2[:])
                tc1 = gpool.tile([128, GRP, 168], BF, tag="tc1")
                nc.scalar.activation(tc1[:], c1t[:], TANHF)
                # h1 split into xbar-transposable contiguous pieces:
                # main rows 0:128 and tail rows 128:168 (tail padded to a
                # 128-col tile; cols 40:128 are never-read garbage).
                h1m = tpool.tile([128, GRP, 128], BF, tag="h1m")
                nc.vector.tensor_mul(h1m[:], s_o[:, :, 0:128], tc1[:, :, 0:128])
                h1tl = tpool.tile([128, GRP, 128], BF, tag="h1tl")
                nc.vector.tensor_mul(h1tl[:, :, 0:40], s_o[:, :, 128:168],
                                     tc1[:, :, 128:168])
                # one batched 7x(128x128) xbar transpose per group for the
                # main rows, one for the padded tail (both sync ring), then
                # one grouped cast tail -> fp8 K8 plane 1 (gpsimd so its
                # DMA-completion wait does not head-block the vector queue).
                nc.sync.dma_start_transpose(
                    Bt[:].rearrange("p (j c) -> p j c", j=GRP), h1m[:])
                tT = tpool.tile([128, GRP, 128], BF, tag="tT")
                nc.sync.dma_start_transpose(tT[:], h1tl[:])
                nc.gpsimd.tensor_copy(K8[0:40, 1, :],
                                      tT[0:40].rearrange("p j c -> p (j c)"))
                gs["c1"] = c1t

            def node_algebra(g, sga):
                st = sstate[g]
                CQt = st["CQt"]
                c1t = gstate[g]["c1"]
                OUTt = opool.tile([128, GRP, 336], BF, tag="OUT")
                s_i2 = sga[:, :, 840:1008]
                s_o2 = sga[:, :, 1008:1176]
                s_fl2 = sga[:, :, 1176:1344]
                s_fd2 = sga[:, :, 1344:1512]
                s_u2 = sga[:, :, 1512:1680]
                tu2 = gpool.tile([128, GRP, 168], BF, tag="tu2")
                nc.gpsimd.tensor_scalar(tu2[:], s_u2, 2.0, -1.0, MULT, ADD)
                m3 = gpool.tile([128, GRP, 168], BF, tag="tmpA")
                nc.vector.tensor_mul(m3[:], s_i2, tu2[:])
                m4 = gpool.tile([128, GRP, 168], BF, tag="tmpB")
                nc.vector.tensor_mul(m4[:], s_fd2, CQt[:, :, 168:336])
                m5 = gpool.tile([128, GRP, 168], BF, tag="tmpC")
                nc.vector.tensor_mul(m5[:], s_fl2, c1t[:])
                a1 = gpool.tile([128, GRP, 168], BF, tag="tmpA")
                nc.vector.tensor_add(a1[:], m3[:], m4[:])
                nc.vector.tensor_add(OUTt[:, :, 168:336], a1[:], m5[:])
                tc2 = gpool.tile([128, GRP, 168], BF, tag="tc2")
                nc.scalar.activation(tc2[:], OUTt[:, :, 168:336], TANHF)
                nc.vector.tensor_mul(OUTt[:, :, 0:168], s_o2, tc2[:])
                return OUTt

            def store_group(g, OUTt):
                cs = g * GNODES
                nc.gpsimd.dma_start(
                    out_d[cs:cs + GNODES, :].rearrange("(m p) f -> p m f",
                                                       p=128),
                    OUTt[:])

            load_group_main(0, split=True)
            if ngrp > 1:
                load_group_main(1, split=True)
            # zero the leaf-bank garbage windows (cols 336:420 per leaf
            # bank) of both fused PSUM buffers so the rectangular merged
            # ACT never reads uninitialized PSUM.
            warm = None
            for _ in range(2):
                Pz = fpool.tile([128, 2048], F32, tag="P")
                nc.vector.memset(Pz[:, 336:420], 0.0)
                nc.vector.memset(Pz[:, 848:932], 0.0)
                # bank-3 tail, never used by real matmuls: HAM warm target
                warm = Pz[:, 1980:2044]
            # warm up the PE HAM clock gate with dummy matmuls while the
            # first input tiles stream in (cold PE runs at half clock).
            for _ in range(110):
                nc.tensor.matmul(warm, ident[:], ident[:, 0:64],
                                 start=True, stop=True)
            for g in range(ngrp + NL):
                gl = g if g < ngrp else None
                gn = g - NL if g >= NL else None
                if g == 0:
                    load_group_cq(0)
                sga = fused_block(gl, gn)
                if gl is not None and gl < 2:
                    # keep the PE HAM warm through the startup load waits;
                    # reading this group's sigmoid output pins these after
                    # the leaf block (else the scheduler hoists them early)
                    for _ in range(120 if g == 0 else 60):
                        nc.tensor.matmul(warm, ident[:], sga[:, 0, 0:64],
                                         start=True, stop=True)
                if gn is not None:
                    store_group(gn, node_algebra(gn, sga))
                if gl is not None and gl + 2 < ngrp:
                    load_group_main(gl + 2)
                if gl is not None and gl + 1 < ngrp:
                    load_group_cq(gl + 1)
                if gl is not None:
                    chain_block(gl, sga)

    nc.compile()
    return nc


def _q8(a):
    return np.clip(np.asarray(a, np.float32), -240, 240).astype(F8)


def _prep_core(inputs, c, npad=NPAD, nper=NPER):
    """Build the per-core (sharded, transposed, fp8/bf16) input arrays."""
    sl = slice(c * nper, (c + 1) * nper)
    e = inputs["e"][sl]
    h_prev = inputs["h_prev"][sl]
    tag = inputs["tag"][sl]
    tagp = inputs["tag_parent"][sl]
    k = inputs["k"][sl]
    c_prev = inputs["c_prev"][sl]
    q = inputs["q"][sl]
    n = e.shape[0]

    x8 = np.zeros((512, npad), F8)
    x8[0:300, :n] = _q8(e.T)
    x8[300:468, :n] = _q8(h_prev.T)
    k8 = np.zeros((256, npad), F8)
    k8[0:128, :n] = _q8(k[:, 0:128].T)
    # plane 1: partitions 0:40 are the device-written h1 tail slot;
    # 40:80 carry k tail rows; rest zero.
    k8[168:208, :n] = _q8(k[:, 128:168].T)
    tt1T = np.zeros((101, npad), BF16)
    tt1T[0:50, :n] = tag.T
    tt1T[50:100, :n] = tagp.T
    tt1T[100, :n] = 1.0
    cq = np.zeros((npad, 336), BF16)
    cq[:n, 0:168] = c_prev
    cq[:n, 168:336] = q
    return dict(x8=x8, k8=k8, tt1T=tt1T, cq=cq)


def _prep_weights(inputs):
    cat = np.concatenate
    # leaf fused weights [569, 672]: rows [e(300); h_prev(168); tags(101)],
    # gate cols [i,o,fl,u]; u columns pre-scaled x2 (tanh via sigmoid),
    # everything x WS for fp8 range (sigmoid applies 1/WS).
    w_l = cat([inputs["We_l"], inputs["Uh_l"],
               inputs["Wt_l"], inputs["Wtp_l"],
               inputs["b_l"][None, :]], 0).astype(np.float64) * WS
    w_l[:, 504:672] *= 2.0
    # node fused [438, 840]: rows [tags(101); h1(168); k(168)],
    # gate cols [i,o,fl,fd,u] (u cols x2); Uk covers gates 0:672 only.
    w_n = np.zeros((437, 840), np.float64)
    w_n[0:50] = inputs["Wt_n"]
    w_n[50:100] = inputs["Wtp_n"]
    w_n[100] = inputs["b_n"]
    w_n[101:269] = inputs["Uh_n"]
    w_n[269:437, 0:672] = inputs["Uk_n"]
    w_n *= WS
    w_n[:, 672:840] *= 2.0

    w8 = np.zeros((128, W8_COLS), F8)

    def put8(base, stride, n, pl0, pl1):
        # pl0/pl1: [rows<=128, n] weight blocks for the two DR planes
        for i, pl in enumerate((pl0, pl1)):
            w8[0:pl.shape[0], base + i * stride:base + i * stride + n] = \
                _q8(pl)

    # leaf pair 1: planes e[0:128], e[128:256]
    put8(W8_L1A, 336, 336, w_l[0:128, 0:336], w_l[128:256, 0:336])
    put8(W8_L1B, 336, 336, w_l[0:128, 336:672], w_l[128:256, 336:672])
    # leaf pair 2: planes [e256:300;h0:84], [h84:168; pad44]
    put8(W8_L2A, 336, 336, w_l[256:384, 0:336], w_l[384:468, 0:336])
    put8(W8_L2B, 336, 336, w_l[256:384, 336:672], w_l[384:468, 336:672])
    # node k pair: plane0 = k rows 0:128 (Uk), plane1 = [h1 tail rows
    # 128:168 (Uh) at 0:40; k tail rows at 40:80]
    kp0 = w_n[269:397]
    kp1 = np.zeros((80, 840))
    kp1[0:40] = w_n[229:269]   # Uh rows 128:168
    kp1[40:80] = w_n[397:437]  # Uk rows 128:168
    put8(W8_KA, 432, 420, kp0[:, 0:420], kp1[:, 0:420])
    put8(W8_KB, 432, 420, kp0[:, 420:840], kp1[:, 420:840])

    wbp = np.zeros((128, WB_COLS), BF16)
    tl = w_l[468:569]          # leaf tags rows [101, 672]
    wbp[0:101, WB_TLA:WB_TLA + 336] = tl[:, 0:336].astype(BF16)
    wbp[0:101, WB_TLB:WB_TLB + 336] = tl[:, 336:672].astype(BF16)
    tn = w_n[0:101]            # node tags rows [101, 840]
    wbp[0:101, WB_TNA:WB_TNA + 420] = tn[:, 0:420].astype(BF16)
    wbp[0:101, WB_TNB:WB_TNB + 420] = tn[:, 420:840].astype(BF16)
    uh = w_n[101:229]          # Uh rows 0:128 [128, 840]
    wbp[:, WB_UHA:WB_UHA + 420] = uh[:, 0:420].astype(BF16)
    wbp[:, WB_UHB:WB_UHB + 420] = uh[:, 420:840].astype(BF16)
    return dict(w8=w8, wb=wbp)


def kernel(**inputs):
    global _compiled, LAST_RESULT
    inputs = {k: np.asarray(v) for k, v in inputs.items()}
    if _compiled is None:
        _compiled = _build()
    weights = _prep_weights(inputs)
    in_maps = []
    for c in range(N_CORES):
        m = _prep_core(inputs, c)
        m.update(weights)
        in_maps.append(m)
    res = run_bass_kernel_spmd(_compiled, in_maps,
                               core_ids=list(range(N_CORES)))
    LAST_RESULT = res
    outs = [res.results[c]["out"][:NPER].astype(np.float32)
            for c in range(N_CORES)]
    return np.concatenate(outs, 0)



# revision 24
# speedup vs baseline: 1.1247x; 1.0438x over previous
"""Trainium2 Bass kernel for the batched ConstituencyTreeLSTM cell.

Data-parallel across 8 NeuronCores: each core processes 12500 nodes
(padded to 12544 = 98 micro-tiles of 128 nodes, 7 per group).

fp8 (e4m3, TRN ±240) DoubleRow matmuls carry the large input chunks:
  leaf:  [e;h_prev] as two DR pairs (4 planes x 128 rows, K_eff=256) vs
         fp8 weights; tags+bias chunk stays bf16.  PSUM [128, i|o|fl|u],
         all four gates evaluated with ONE sigmoid per micro-tile
         (u-gate weights pre-scaled x2 so tanh(u) = 2*sig(2u)-1).
  node:  tags chunk + Uh(h1^T rows 0:128, via DMA xbar transpose) in
         bf16; DR pair (k[0:128] | [h1^T tail rows; k tail rows]) in fp8
         (h1 tail transposed on PE, cast to fp8 by the DVE evac copy).
         PSUM [128, i2|o2|fl2|fd2|u2], one sigmoid per micro-tile.
All matmul weights are pre-scaled x64 (fp8 range use / denormal
avoidance); the sigmoid activation applies scale=1/64 for free.
tanh(c1)/tanh(c2) run per-group on ACT; gate algebra on DVE with the
2s-1 affine on GPSIMD.  Pipeline: node block of group g runs two
iterations after its leaf block (keeps PE HAM warm through loads).
"""

import os
import sys

import numpy as np

try:
    import concourse.bass as bass  # noqa: F401
except Exception:  # pragma: no cover - fallback for bare environments
    for p in (
        "/root/.axon_site",
        "/root/.axon_site/_ro/trn_rl_repo",
        "/root/.axon_site/_ro/pypackages",
        "/opt/trn_rl_repo",
        "/opt/pypackages",
    ):
        if os.path.isdir(p) and p not in sys.path:
            sys.path.append(p)
    import concourse.bass as bass  # noqa: F401

import ml_dtypes
import concourse.mybir as mybir
import concourse.tile as tile
from concourse import bacc
from concourse.bass_utils import run_bass_kernel_spmd
from concourse.masks import make_identity

BF16 = ml_dtypes.bfloat16
F8 = ml_dtypes.float8_e4m3  # TRN fp8e4: max +-240, S.1111.000 = inf

N_CORES = 8
N = 100000
NPER = N // N_CORES            # 12500
MICRO = 128                    # nodes per matmul tile
GRP = 7                        # micro tiles per group
GNODES = MICRO * GRP           # 896
NGRP = 14                      # groups per core
NPAD = NGRP * GNODES           # 12544
M = 168                        # mem dim
NL = 3                         # leaf->node pipeline lag (groups)

WS = 64.0                      # fp8 weight pre-scale (1/WS folded into sig)

F32 = mybir.dt.float32
BF = mybir.dt.bfloat16
FP8 = mybir.dt.float8e4
SIGF = mybir.ActivationFunctionType.Sigmoid
TANHF = mybir.ActivationFunctionType.Tanh
DR = mybir.MatmulPerfMode.DoubleRow
MULT = mybir.AluOpType.mult
ADD = mybir.AluOpType.add

# fp8 weight pack column layout (1B/col): leaf DR pairs (2 planes x 336)
# and the node k-pair (2 planes x 420, plane stride padded to 432).
W8_L1A, W8_L1B = 0, 672
W8_L2A, W8_L2B = 1344, 2016
W8_KA, W8_KB = 2688, 3552
W8_COLS = 4416
# bf16 weight pack: leaf tags (2x336), node tags (2x420), node Uh (2x420)
WB_TLA, WB_TLB = 0, 336
WB_TNA, WB_TNB = 672, 1092
WB_UHA, WB_UHB = 1512, 1932
WB_COLS = 2352

_compiled = None
LAST_RESULT = None


def _build(ngrp=NGRP):
    npad = ngrp * GNODES
    nc = bacc.Bacc("TRN2", target_bir_lowering=False, debug=False,
                   num_devices=N_CORES)

    x8_d = nc.dram_tensor("x8", [512, npad], FP8, kind="ExternalInput")
    k8_d = nc.dram_tensor("k8", [256, npad], FP8, kind="ExternalInput")
    tt1T_d = nc.dram_tensor("tt1T", [101, npad], BF, kind="ExternalInput")
    cq_d = nc.dram_tensor("cq", [npad, 336], BF, kind="ExternalInput")
    w8_d = nc.dram_tensor("w8", [128, W8_COLS], FP8, kind="ExternalInput")
    wb_d = nc.dram_tensor("wb", [128, WB_COLS], BF, kind="ExternalInput")
    out_d = nc.dram_tensor("out", [npad, 336], BF, kind="ExternalOutput")

    with tile.TileContext(nc) as tc:
        from contextlib import ExitStack
        with ExitStack() as ctx:
            wpool = ctx.enter_context(tc.tile_pool(name="w", bufs=1))
            epool = ctx.enter_context(tc.tile_pool(name="e", bufs=3))
            npool = ctx.enter_context(tc.tile_pool(name="n", bufs=NL + 3))
            opool = ctx.enter_context(tc.tile_pool(name="o", bufs=2))
            gpool = ctx.enter_context(tc.tile_pool(name="g", bufs=2))
            c1pool = ctx.enter_context(tc.tile_pool(name="c1p", bufs=NL + 1))
            tpool = ctx.enter_context(tc.tile_pool(name="tp", bufs=2))
            fpool = ctx.enter_context(
                tc.tile_pool(name="fp", bufs=2, space="PSUM"))

            # ---- constants / weights (resident) ----
            ident = wpool.tile([128, 128], BF, tag="ident")
            make_identity(nc, ident[:])
            w8 = wpool.tile([128, W8_COLS], FP8, tag="w8")
            nc.sync.dma_start(w8[:], w8_d[:, :])
            wb = wpool.tile([128, WB_COLS], BF, tag="wb")
            nc.sync.dma_start(wb[:], wb_d[:, :])

            def w8r(base, n, stride):
                blk = w8[:, base:base + 2 * stride]
                return blk.rearrange("p (a b) -> p a b", a=2,
                                     b=stride)[:, :, 0:n]

            sstate = {}
            gstate = {}

            def load_group_main(g, split=False):
                cs = g * GNODES
                EK = epool.tile([128, 4, GNODES], FP8, tag="EK")
                K8 = npool.tile([128, 2, GNODES], FP8, tag="K8")
                TTs = npool.tile([128, GNODES], BF, tag="TT")
                h = GNODES // 2
                rngs = ((0, h), (h, GNODES)) if split else ((0, GNODES),)
                # startup groups: spread the load burst across both rings
                eng2 = nc.gpsimd if split else nc.sync
                for c0, c1 in rngs:
                    nc.sync.dma_start(
                        EK[:, :, c0:c1],
                        x8_d[:, cs + c0:cs + c1].rearrange(
                            "(c p) n -> p c n", p=128))
                    eng2.dma_start(
                        K8[:, :, c0:c1],
                        k8_d[:, cs + c0:cs + c1].rearrange(
                            "(c p) n -> p c n", p=128))
                    eng2.dma_start(TTs[0:101, c0:c1],
                                   tt1T_d[:, cs + c0:cs + c1])
                Bt = npool.tile([128, GNODES], BF, tag="B")
                sstate[g] = dict(EK=EK, K8=K8, TT=TTs, B=Bt)

            def load_group_cq(g):
                cs = g * GNODES
                CQt = npool.tile([128, GRP, 336], BF, tag="CQ")
                nc.gpsimd.dma_start(
                    CQt[:],
                    cq_d[cs:cs + GNODES, :].rearrange("(m p) f -> p m f",
                                                      p=128))
                sstate[g].update(CQt=CQt)

            def fused_block(gl, gn, hooks):
                """Interleaved leaf(gl) + node(gn) matmuls, one merged
                sigmoid ACT per micro-tile over the 4-bank fused PSUM tile.
                sga flat layout per micro: [i,o,x84,fl,u,x84 | i2,o2,fl2,fd2,u2]
                (x84 = garbage from the rectangular [., 420] ACT window)."""
                sga = gpool.tile([128, GRP, 1680], BF, tag="sga")
                stl = sstate[gl] if gl is not None else None
                stn = sstate[gn] if gn is not None else None
                for j in range(GRP):
                    c0 = j * MICRO
                    P = fpool.tile([128, 2048], F32, tag="P")
                    if gl is not None:
                        EK, TTs = stl["EK"], stl["TT"]
                        XA = EK[:, 0:2, c0:c0 + MICRO]
                        XB = EK[:, 2:4, c0:c0 + MICRO]
                        XT = TTs[0:101, c0:c0 + MICRO]
                        nc.tensor.matmul(P[:, 0:336], XA, w8r(W8_L1A, 336, 336),
                                         start=True, stop=False, perf_mode=DR)
                        nc.tensor.matmul(P[:, 512:848], XA, w8r(W8_L1B, 336, 336),
                                         start=True, stop=False, perf_mode=DR)
                        nc.tensor.matmul(P[:, 0:336], XB, w8r(W8_L2A, 336, 336),
                                         start=False, stop=False, perf_mode=DR)
                        nc.tensor.matmul(P[:, 512:848], XB, w8r(W8_L2B, 336, 336),
                                         start=False, stop=False, perf_mode=DR)
                        nc.tensor.matmul(P[:, 0:336], XT,
                                         wb[0:101, WB_TLA:WB_TLA + 336],
                                         start=False, stop=True)
                        nc.tensor.matmul(P[:, 512:848], XT,
                                         wb[0:101, WB_TLB:WB_TLB + 336],
                                         start=False, stop=True)
                    if gn is not None:
                        TTn, Btn, K8n = stn["TT"], stn["B"], stn["K8"]
                        XTn = TTn[0:101, c0:c0 + MICRO]
                        XBn = Btn[:, c0:c0 + MICRO]
                        XKn = K8n[:, 0:2, c0:c0 + MICRO]
                        nc.tensor.matmul(P[:, 1024:1444], XTn,
                                         wb[0:101, WB_TNA:WB_TNA + 420],
                                         start=True, stop=False)
                        nc.tensor.matmul(P[:, 1536:1956], XTn,
                                         wb[0:101, WB_TNB:WB_TNB + 420],
                                         start=True, stop=False)
                        nc.tensor.matmul(P[:, 1024:1444], XBn,
                                         wb[:, WB_UHA:WB_UHA + 420],
                                         start=False, stop=False)
                        nc.tensor.matmul(P[:, 1536:1956], XBn,
                                         wb[:, WB_UHB:WB_UHB + 420],
                                         start=False, stop=False)
                        nc.tensor.matmul(P[:, 1024:1444], XKn,
                                         w8r(W8_KA, 420, 432),
                                         start=False, stop=True, perf_mode=DR)
                        nc.tensor.matmul(P[:, 1536:1956], XKn,
                                         w8r(W8_KB, 420, 432),
                                         start=False, stop=True, perf_mode=DR)
                    pr = P[:].rearrange("p (a b) -> p a b", a=4, b=512)
                    sr = sga[:, j, :].rearrange("p (a b) -> p a b", a=4, b=420)
                    if gl is not None and gn is not None:
                        nc.scalar.activation(sr, pr[:, :, 0:420], SIGF,
                                             scale=1.0 / WS)
                    elif gl is not None:
                        nc.scalar.activation(sr[:, 0:2], pr[:, 0:2, 0:420],
                                             SIGF, scale=1.0 / WS)
                    else:
                        nc.scalar.activation(sr[:, 2:4], pr[:, 2:4, 0:420],
                                             SIGF, scale=1.0 / WS)
                    # deferred cross-group ACT work, interleaved between
                    # sigmoids so the scalar FIFO never head-blocks on the
                    # previous group's DVE chains
                    for fn in hooks.get(j, ()):
                        fn()
                if gl is not None:
                    gstate[gl] = dict()
                return sga

            # half-group split points for the deferred tanh ACTs
            HA, HB = 4, GRP  # halves [0:4) and [4:7)

            def tanh_c1(g, half):
                """Deferred: tanh of half of c1(g), into gstate[g]['tc1']."""
                gs = gstate[g]
                if "tc1" not in gs:
                    gs["tc1"] = gpool.tile([128, GRP, 168], BF, name="tc1", tag="tc1")
                lo, hi = (0, HA) if half == 0 else (HA, HB)
                nc.scalar.activation(gs["tc1"][:, lo:hi], gs["c1"][:, lo:hi],
                                     TANHF)

            def tanh_c2(g, half):
                gs = gstate[g]
                if "tc2" not in gs:
                    gs["tc2"] = gpool.tile([128, GRP, 168], BF, name="tc2", tag="tc2")
                lo, hi = (0, HA) if half == 0 else (HA, HB)
                nc.scalar.activation(gs["tc2"][:, lo:hi],
                                     gs["OUT"][:, lo:hi, 168:336], TANHF)

            def chain_fin(g):
                """h1 = sig(o)*tanh(c1) and its batched transposes (iter
                g+1): Bt main rows via one 7x(128x128) xbar, tail rows via
                a [128,8,64]-padded xbar; the fp8 cast into K8 plane 1 is
                emitted later (cast_fin) so its DMA wait sits at the very
                end of the gpsimd queue."""
                st = sstate[g]
                gs = gstate[g]
                sga, tc1 = gs["sga"], gs["tc1"]
                s_o = sga[:, :, 168:336]
                h1m = tpool.tile([128, GRP, 128], BF, tag="h1m")
                nc.vector.tensor_mul(h1m[:], s_o[:, :, 0:128], tc1[:, :, 0:128])
                h1tl = tpool.tile([128, GRP, 128], BF, tag="h1tl")
                nc.vector.tensor_mul(
                    h1tl[:, :, 0:40],
                    s_o[:, :, 128:168], tc1[:, :, 128:168])
                nc.sync.dma_start_transpose(
                    st["B"][:].rearrange("p (j c) -> p j c", j=GRP), h1m[:])
                tT = tpool.tile([128, GRP, 128], BF, tag="tT")
                nc.sync.dma_start_transpose(tT[:], h1tl[:])
                gs["tT"] = tT

            def cast_fin(g):
                st = sstate[g]
                gs = gstate[g]
                tT = gs["tT"]
                nc.gpsimd.tensor_copy(
                    st["K8"][0:40, 1, :],
                    tT[0:40].rearrange("p j c -> p (j c)"))

            def node_pre(g, sga):
                """c2(g) gate algebra (vector), after the group's sigs."""
                st = sstate[g]
                gs = gstate[g]
                CQt = st["CQt"]
                c1t = gs["c1"]
                OUTt = opool.tile([128, GRP, 336], BF, tag="OUT")
                s_i2 = sga[:, :, 840:1008]
                s_fl2 = sga[:, :, 1176:1344]
                s_fd2 = sga[:, :, 1344:1512]
                s_u2 = sga[:, :, 1512:1680]
                tu2 = gpool.tile([128, GRP, 168], BF, tag="tu2")
                nc.gpsimd.tensor_scalar(tu2[:], s_u2, 2.0, -1.0, MULT, ADD)
                m3 = gpool.tile([128, GRP, 168], BF, tag="tmpA")
                nc.vector.tensor_mul(m3[:], s_i2, tu2[:])
                m4 = gpool.tile([128, GRP, 168], BF, tag="tmpB")
                nc.vector.tensor_mul(m4[:], s_fd2, CQt[:, :, 168:336])
                m5 = gpool.tile([128, GRP, 168], BF, tag="tmpC")
                nc.vector.tensor_mul(m5[:], s_fl2, c1t[:])
                a1 = gpool.tile([128, GRP, 168], BF, tag="tmpA")
                nc.vector.tensor_add(a1[:], m3[:], m4[:])
                nc.vector.tensor_add(OUTt[:, :, 168:336], a1[:], m5[:])
                gs["OUT"] = OUTt
                gs["s_o2"] = sga[:, :, 1008:1176]

            def node_fin(g):
                """h2 = sig(o2)*tanh(c2) (iter g+1) + store."""
                gs = gstate[g]
                OUTt = gs["OUT"]
                nc.vector.tensor_mul(OUTt[:, :, 0:168], gs["s_o2"],
                                     gs["tc2"][:])
                cs = g * GNODES
                nc.gpsimd.dma_start(
                    out_d[cs:cs + GNODES, :].rearrange("(m p) f -> p m f",
                                                       p=128),
                    OUTt[:])

            def chain_pre(g, sga):
                """c1(g) gate algebra (vector), after the group's sigs."""
                st = sstate[g]
                gs = gstate[g]
                CQt = st["CQt"]
                s_i = sga[:, :, 0:168]
                s_fl = sga[:, :, 420:588]
                s_u = sga[:, :, 588:756]
                tu = gpool.tile([128, GRP, 168], BF, tag="tu")
                nc.gpsimd.tensor_scalar(tu[:], s_u, 2.0, -1.0, MULT, ADD)
                m1 = gpool.tile([128, GRP, 168], BF, tag="tmpA")
                nc.vector.tensor_mul(m1[:], s_i, tu[:])
                m2 = gpool.tile([128, GRP, 168], BF, tag="tmpB")
                nc.vector.tensor_mul(m2[:], s_fl, CQt[:, :, 0:168])
                c1t = c1pool.tile([128, GRP, 168], BF, tag="c1")
                nc.vector.tensor_add(c1t[:], m1[:], m2[:])
                gs["c1"] = c1t
                gs["sga"] = sga

            load_group_main(0, split=True)
            if ngrp > 1:
                load_group_main(1, split=True)
            # zero the leaf-bank garbage windows (cols 336:420 per leaf
            # bank) of both fused PSUM buffers so the rectangular merged
            # ACT never reads uninitialized PSUM.
            warm = None
            for _ in range(2):
                Pz = fpool.tile([128, 2048], F32, tag="P")
                nc.vector.memset(Pz[:, 336:420], 0.0)
                nc.vector.memset(Pz[:, 848:932], 0.0)
                # bank-3 tail, never used by real matmuls: HAM warm target
                warm = Pz[:, 1980:2044]
            # warm up the PE HAM clock gate with dummy matmuls while the
            # first input tiles stream in (cold PE runs at half clock).
            for _ in range(110):
                nc.tensor.matmul(warm, ident[:], ident[:, 0:64],
                                 start=True, stop=True)
            for g in range(ngrp + NL + 1):
                gl = g if g < ngrp else None
                gn = g - NL if NL <= g < ngrp + NL else None
                gc = g - 1 if 0 <= g - 1 < ngrp else None
                gp = g - 1 - NL if 0 <= g - 1 - NL < ngrp else None
                hooks = {}
                if gc is not None:
                    hooks.setdefault(1, []).append(
                        lambda gg=gc: tanh_c1(gg, 0))
                    hooks.setdefault(2, []).append(
                        lambda gg=gc: tanh_c1(gg, 1))
                if gp is not None:
                    hooks.setdefault(3, []).append(
                        lambda gg=gp: tanh_c2(gg, 0))
                    hooks.setdefault(4, []).append(
                        lambda gg=gp: tanh_c2(gg, 1))
                if g == 0:
                    load_group_cq(0)
                if gl is not None or gn is not None:
                    sga = fused_block(gl, gn, hooks)
                else:
                    for j in sorted(hooks):
                        for fn in hooks[j]:
                            fn()
                    sga = None
                if gl is not None and gl < 2:
                    # keep the PE HAM warm through the startup load waits;
                    # reading this group's sigmoid output pins these after
                    # the leaf block (else the scheduler hoists them early)
                    for _ in range(120 if g == 0 else 60):
                        nc.tensor.matmul(warm, ident[:], sga[:, 0, 0:64],
                                         start=True, stop=True)
                if gc is not None:
                    chain_fin(gc)
                if gp is not None:
                    node_fin(gp)
                if gn is not None:
                    node_pre(gn, sga)
                if gl is not None and gl + 2 < ngrp:
                    load_group_main(gl + 2)
                if gl is not None and gl + 1 < ngrp:
                    load_group_cq(gl + 1)
                if gl is not None:
                    chain_pre(gl, sga)
                if gc is not None:
                    cast_fin(gc)

    nc.compile()
    return nc


def _q8(a):
    return np.clip(np.asarray(a, np.float32), -240, 240).astype(F8)


def _prep_core(inputs, c, npad=NPAD, nper=NPER):
    """Build the per-core (sharded, transposed, fp8/bf16) input arrays."""
    sl = slice(c * nper, (c + 1) * nper)
    e = inputs["e"][sl]
    h_prev = inputs["h_prev"][sl]
    tag = inputs["tag"][sl]
    tagp = inputs["tag_parent"][sl]
    k = inputs["k"][sl]
    c_prev = inputs["c_prev"][sl]
    q = inputs["q"][sl]
    n = e.shape[0]

    x8 = np.zeros((512, npad), F8)
    x8[0:300, :n] = _q8(e.T)
    x8[300:468, :n] = _q8(h_prev.T)
    k8 = np.zeros((256, npad), F8)
    k8[0:128, :n] = _q8(k[:, 0:128].T)
    # plane 1: partitions 0:40 are the device-written h1 tail slot;
    # 40:80 carry k tail rows; rest zero.
    k8[168:208, :n] = _q8(k[:, 128:168].T)
    tt1T = np.zeros((101, npad), BF16)
    tt1T[0:50, :n] = tag.T
    tt1T[50:100, :n] = tagp.T
    tt1T[100, :n] = 1.0
    cq = np.zeros((npad, 336), BF16)
    cq[:n, 0:168] = c_prev
    cq[:n, 168:336] = q
    return dict(x8=x8, k8=k8, tt1T=tt1T, cq=cq)


def _prep_weights(inputs):
    cat = np.concatenate
    # leaf fused weights [569, 672]: rows [e(300); h_prev(168); tags(101)],
    # gate cols [i,o,fl,u]; u columns pre-scaled x2 (tanh via sigmoid),
    # everything x WS for fp8 range (sigmoid applies 1/WS).
    w_l = cat([inputs["We_l"], inputs["Uh_l"],
               inputs["Wt_l"], inputs["Wtp_l"],
               inputs["b_l"][None, :]], 0).astype(np.float64) * WS
    w_l[:, 504:672] *= 2.0
    # node fused [438, 840]: rows [tags(101); h1(168); k(168)],
    # gate cols [i,o,fl,fd,u] (u cols x2); Uk covers gates 0:672 only.
    w_n = np.zeros((437, 840), np.float64)
    w_n[0:50] = inputs["Wt_n"]
    w_n[50:100] = inputs["Wtp_n"]
    w_n[100] = inputs["b_n"]
    w_n[101:269] = inputs["Uh_n"]
    w_n[269:437, 0:672] = inputs["Uk_n"]
    w_n *= WS
    w_n[:, 672:840] *= 2.0

    w8 = np.zeros((128, W8_COLS), F8)

    def put8(base, stride, n, pl0, pl1):
        # pl0/pl1: [rows<=128, n] weight blocks for the two DR planes
        for i, pl in enumerate((pl0, pl1)):
            w8[0:pl.shape[0], base + i * stride:base + i * stride + n] = \
                _q8(pl)

    # leaf pair 1: planes e[0:128], e[128:256]
    put8(W8_L1A, 336, 336, w_l[0:128, 0:336], w_l[128:256, 0:336])
    put8(W8_L1B, 336, 336, w_l[0:128, 336:672], w_l[128:256, 336:672])
    # leaf pair 2: planes [e256:300;h0:84], [h84:168; pad44]
    put8(W8_L2A, 336, 336, w_l[256:384, 0:336], w_l[384:468, 0:336])
    put8(W8_L2B, 336, 336, w_l[256:384, 336:672], w_l[384:468, 336:672])
    # node k pair: plane0 = k rows 0:128 (Uk), plane1 = [h1 tail rows
    # 128:168 (Uh) at 0:40; k tail rows at 40:80]
    kp0 = w_n[269:397]
    kp1 = np.zeros((80, 840))
    kp1[0:40] = w_n[229:269]   # Uh rows 128:168
    kp1[40:80] = w_n[397:437]  # Uk rows 128:168
    put8(W8_KA, 432, 420, kp0[:, 0:420], kp1[:, 0:420])
    put8(W8_KB, 432, 420, kp0[:, 420:840], kp1[:, 420:840])

    wbp = np.zeros((128, WB_COLS), BF16)
    tl = w_l[468:569]          # leaf tags rows [101, 672]
    wbp[0:101, WB_TLA:WB_TLA + 336] = tl[:, 0:336].astype(BF16)
    wbp[0:101, WB_TLB:WB_TLB + 336] = tl[:, 336:672].astype(BF16)
    tn = w_n[0:101]            # node tags rows [101, 840]
    wbp[0:101, WB_TNA:WB_TNA + 420] = tn[:, 0:420].astype(BF16)
    wbp[0:101, WB_TNB:WB_TNB + 420] = tn[:, 420:840].astype(BF16)
    uh = w_n[101:229]          # Uh rows 0:128 [128, 840]
    wbp[:, WB_UHA:WB_UHA + 420] = uh[:, 0:420].astype(BF16)
    wbp[:, WB_UHB:WB_UHB + 420] = uh[:, 420:840].astype(BF16)
    return dict(w8=w8, wb=wbp)


def kernel(**inputs):
    global _compiled, LAST_RESULT
    inputs = {k: np.asarray(v) for k, v in inputs.items()}
    if _compiled is None:
        _compiled = _build()
    weights = _prep_weights(inputs)
    in_maps = []
    for c in range(N_CORES):
        m = _prep_core(inputs, c)
        m.update(weights)
        in_maps.append(m)
    res = run_bass_kernel_spmd(_compiled, in_maps,
                               core_ids=list(range(N_CORES)))
    LAST_RESULT = res
    outs = [res.results[c]["out"][:NPER].astype(np.float32)
            for c in range(N_CORES)]
    return np.concatenate(outs, 0)



# revision 26
# speedup vs baseline: 1.1527x; 1.0249x over previous
"""Trainium2 Bass kernel for the batched ConstituencyTreeLSTM cell.

Data-parallel across 8 NeuronCores: each core processes 12500 nodes
(padded to 12544 = 98 micro-tiles of 128 nodes, 7 per group).

fp8 (e4m3, TRN ±240) DoubleRow matmuls carry the large input chunks:
  leaf:  [e;h_prev] as two DR pairs (4 planes x 128 rows, K_eff=256) vs
         fp8 weights; tags+bias chunk stays bf16.  PSUM [128, i|o|fl|u],
         all four gates evaluated with ONE sigmoid per micro-tile
         (u-gate weights pre-scaled x2 so tanh(u) = 2*sig(2u)-1).
  node:  tags chunk + Uh(h1^T rows 0:128, via DMA xbar transpose) in
         bf16; DR pair (k[0:128] | [h1^T tail rows; k tail rows]) in fp8
         (h1 tail transposed on PE, cast to fp8 by the DVE evac copy).
         PSUM [128, i2|o2|fl2|fd2|u2], one sigmoid per micro-tile.
All matmul weights are pre-scaled x64 (fp8 range use / denormal
avoidance); the sigmoid activation applies scale=1/64 for free.
tanh(c1)/tanh(c2) run per-group on ACT; gate algebra on DVE with the
2s-1 affine on GPSIMD.  Pipeline: node block of group g runs two
iterations after its leaf block (keeps PE HAM warm through loads).
"""

import os
import sys

import numpy as np

try:
    import concourse.bass as bass  # noqa: F401
except Exception:  # pragma: no cover - fallback for bare environments
    for p in (
        "/root/.axon_site",
        "/root/.axon_site/_ro/trn_rl_repo",
        "/root/.axon_site/_ro/pypackages",
        "/opt/trn_rl_repo",
        "/opt/pypackages",
    ):
        if os.path.isdir(p) and p not in sys.path:
            sys.path.append(p)
    import concourse.bass as bass  # noqa: F401

import ml_dtypes
import concourse.mybir as mybir
import concourse.tile as tile
from concourse import bacc
from concourse.bass_utils import run_bass_kernel_spmd
from concourse.masks import make_identity

BF16 = ml_dtypes.bfloat16
F8 = ml_dtypes.float8_e4m3  # TRN fp8e4: max +-240, S.1111.000 = inf

N_CORES = 8
N = 100000
NPER = N // N_CORES            # 12500
MICRO = 128                    # nodes per matmul tile
GRP = 7                        # micro tiles per group
GNODES = MICRO * GRP           # 896
NGRP = 14                      # groups per core
NPAD = NGRP * GNODES           # 12544
M = 168                        # mem dim
NL = 3                         # leaf->node pipeline lag (groups)

WS = 64.0                      # fp8 weight pre-scale (1/WS folded into sig)

F32 = mybir.dt.float32
BF = mybir.dt.bfloat16
FP8 = mybir.dt.float8e4
SIGF = mybir.ActivationFunctionType.Sigmoid
TANHF = mybir.ActivationFunctionType.Tanh
DR = mybir.MatmulPerfMode.DoubleRow
MULT = mybir.AluOpType.mult
ADD = mybir.AluOpType.add

# fp8 weight pack column layout (1B/col): leaf DR pairs (2 planes x 336)
# and the node k-pair (2 planes x 420, plane stride padded to 432).
W8_L1A, W8_L1B = 0, 672
W8_L2A, W8_L2B = 1344, 2016
W8_KA, W8_KB = 2688, 3552
W8_COLS = 4416
# bf16 weight pack: leaf tags (2x336), node tags (2x420), node Uh (2x420)
WB_TLA, WB_TLB = 0, 336
WB_TNA, WB_TNB = 672, 1092
WB_UHA, WB_UHB = 1512, 1932
WB_COLS = 2352

_compiled = None
LAST_RESULT = None


def _build(ngrp=NGRP):
    npad = ngrp * GNODES
    nc = bacc.Bacc("TRN2", target_bir_lowering=False, debug=False,
                   num_devices=N_CORES)

    x8_d = nc.dram_tensor("x8", [512, npad], FP8, kind="ExternalInput")
    k8_d = nc.dram_tensor("k8", [256, npad], FP8, kind="ExternalInput")
    tt1T_d = nc.dram_tensor("tt1T", [101, npad], BF, kind="ExternalInput")
    cq_d = nc.dram_tensor("cq", [npad, 336], BF, kind="ExternalInput")
    w8_d = nc.dram_tensor("w8", [128, W8_COLS], FP8, kind="ExternalInput")
    wb_d = nc.dram_tensor("wb", [128, WB_COLS], BF, kind="ExternalInput")
    out_d = nc.dram_tensor("out", [npad, 336], BF, kind="ExternalOutput")

    with tile.TileContext(nc) as tc:
        from contextlib import ExitStack
        with ExitStack() as ctx:
            wpool = ctx.enter_context(tc.tile_pool(name="w", bufs=1))
            epool = ctx.enter_context(tc.tile_pool(name="e", bufs=3))
            npool = ctx.enter_context(tc.tile_pool(name="n", bufs=NL + 3))
            opool = ctx.enter_context(tc.tile_pool(name="o", bufs=2))
            gpool = ctx.enter_context(tc.tile_pool(name="g", bufs=2))
            c1pool = ctx.enter_context(tc.tile_pool(name="c1p", bufs=NL + 1))
            tpool = ctx.enter_context(tc.tile_pool(name="tp", bufs=2))
            plpool = ctx.enter_context(
                tc.tile_pool(name="pl", bufs=2, space="PSUM"))
            pnpool = ctx.enter_context(
                tc.tile_pool(name="pn", bufs=2, space="PSUM"))

            # ---- constants / weights (resident) ----
            ident = wpool.tile([128, 128], BF, tag="ident")
            make_identity(nc, ident[:])
            w8 = wpool.tile([128, W8_COLS], FP8, tag="w8")
            nc.sync.dma_start(w8[:], w8_d[:, :])
            wb = wpool.tile([128, WB_COLS], BF, tag="wb")
            nc.sync.dma_start(wb[:], wb_d[:, :])

            def w8r(base, n, stride):
                blk = w8[:, base:base + 2 * stride]
                return blk.rearrange("p (a b) -> p a b", a=2,
                                     b=stride)[:, :, 0:n]

            sstate = {}
            gstate = {}

            def load_group_main(g, split=False):
                cs = g * GNODES
                EK = epool.tile([128, 4, GNODES], FP8, tag="EK")
                K8 = npool.tile([128, 2, GNODES], FP8, tag="K8")
                TTs = npool.tile([128, GNODES], BF, tag="TT")
                h = GNODES // 2
                rngs = ((0, h), (h, GNODES)) if split else ((0, GNODES),)
                # startup groups: spread the load burst across both rings
                eng2 = nc.gpsimd if split else nc.sync
                for c0, c1 in rngs:
                    nc.sync.dma_start(
                        EK[:, :, c0:c1],
                        x8_d[:, cs + c0:cs + c1].rearrange(
                            "(c p) n -> p c n", p=128))
                    eng2.dma_start(
                        K8[:, :, c0:c1],
                        k8_d[:, cs + c0:cs + c1].rearrange(
                            "(c p) n -> p c n", p=128))
                    eng2.dma_start(TTs[0:101, c0:c1],
                                   tt1T_d[:, cs + c0:cs + c1])
                Bt = npool.tile([128, GNODES], BF, tag="B")
                sstate[g] = dict(EK=EK, K8=K8, TT=TTs, B=Bt)

            def load_group_cq(g):
                cs = g * GNODES
                CQt = npool.tile([128, GRP, 336], BF, tag="CQ")
                nc.gpsimd.dma_start(
                    CQt[:],
                    cq_d[cs:cs + GNODES, :].rearrange("(m p) f -> p m f",
                                                      p=128))
                sstate[g].update(CQt=CQt)

            def leaf_block(g):
                st = sstate[g]
                EK, TTs = st["EK"], st["TT"]
                sgl = gpool.tile([128, GRP, 672], BF, tag="sgl")
                for j in range(GRP):
                    c0 = j * MICRO
                    P = plpool.tile([128, 1024], F32, tag="psl")
                    XA = EK[:, 0:2, c0:c0 + MICRO]
                    XB = EK[:, 2:4, c0:c0 + MICRO]
                    nc.tensor.matmul(P[:, 0:336], XA, w8r(W8_L1A, 336, 336),
                                     start=True, stop=False, perf_mode=DR)
                    nc.tensor.matmul(P[:, 512:848], XA, w8r(W8_L1B, 336, 336),
                                     start=True, stop=False, perf_mode=DR)
                    nc.tensor.matmul(P[:, 0:336], XB, w8r(W8_L2A, 336, 336),
                                     start=False, stop=False, perf_mode=DR)
                    nc.tensor.matmul(P[:, 512:848], XB, w8r(W8_L2B, 336, 336),
                                     start=False, stop=False, perf_mode=DR)
                    XT = TTs[0:101, c0:c0 + MICRO]
                    nc.tensor.matmul(P[:, 0:336], XT,
                                     wb[0:101, WB_TLA:WB_TLA + 336],
                                     start=False, stop=True)
                    nc.tensor.matmul(P[:, 512:848], XT,
                                     wb[0:101, WB_TLB:WB_TLB + 336],
                                     start=False, stop=True)
                    pr = P[:].rearrange("p (a b) -> p a b", a=2, b=512)
                    sr = sgl[:, j, :].rearrange("p (a b) -> p a b", a=2, b=336)
                    nc.scalar.activation(sr, pr[:, :, 0:336], SIGF,
                                         scale=1.0 / WS)
                gstate[g] = dict(sgl=sgl)

            def chain_block(g):
                st = sstate[g]
                gs = gstate[g]
                sgl = gs["sgl"]
                CQt, Bt, K8 = st["CQt"], st["B"], st["K8"]
                s_i = sgl[:, :, 0:168]
                s_o = sgl[:, :, 168:336]
                s_fl = sgl[:, :, 336:504]
                s_u = sgl[:, :, 504:672]
                tu = gpool.tile([128, GRP, 168], BF, tag="tu")
                nc.gpsimd.tensor_scalar(tu[:], s_u, 2.0, -1.0, MULT, ADD)
                m1 = gpool.tile([128, GRP, 168], BF, tag="tmpA")
                nc.vector.tensor_mul(m1[:], s_i, tu[:])
                m2 = gpool.tile([128, GRP, 168], BF, tag="tmpB")
                nc.vector.tensor_mul(m2[:], s_fl, CQt[:, :, 0:168])
                c1t = c1pool.tile([128, GRP, 168], BF, tag="c1")
                nc.vector.tensor_add(c1t[:], m1[:], m2[:])
                tc1 = gpool.tile([128, GRP, 168], BF, tag="tc1")
                nc.scalar.activation(tc1[:], c1t[:], TANHF)
                # h1 split into xbar-transposable contiguous pieces:
                # main rows 0:128 and tail rows 128:168 (tail padded to a
                # 128-col tile; cols 40:128 are never-read garbage).
                h1m = tpool.tile([128, GRP, 128], BF, tag="h1m")
                nc.vector.tensor_mul(h1m[:], s_o[:, :, 0:128],
                                     tc1[:, :, 0:128])
                h1tl = tpool.tile([128, GRP, 128], BF, tag="h1tl")
                nc.vector.tensor_mul(h1tl[:, :, 0:40], s_o[:, :, 128:168],
                                     tc1[:, :, 128:168])
                # one batched 7x(128x128) xbar transpose per group for the
                # main rows, one for the padded tail (both sync ring); the
                # fp8 cast of the tail into K8 plane 1 is deferred to the
                # end of the iteration (cast_fin) so its DMA-completion
                # wait sits behind all other vector work.
                nc.sync.dma_start_transpose(
                    Bt[:].rearrange("p (j c) -> p j c", j=GRP), h1m[:])
                tT = tpool.tile([128, GRP, 128], BF, tag="tT")
                nc.sync.dma_start_transpose(tT[:], h1tl[:])
                gs["c1"] = c1t
                gs["tT"] = tT

            def cast_fin(g):
                st = sstate[g]
                gs = gstate[g]
                tT = gs["tT"]
                nc.vector.tensor_copy(
                    st["K8"][0:40, 1, :],
                    tT[0:40].rearrange("p j c -> p (j c)"))

            def node_block(g):
                st = sstate[g]
                TTs, Bt, K8 = st["TT"], st["B"], st["K8"]
                CQt = st["CQt"]
                c1t = gstate[g]["c1"]
                OUTt = opool.tile([128, GRP, 336], BF, tag="OUT")
                sgn = gpool.tile([128, GRP, 840], BF, tag="sgn")
                for j in range(GRP):
                    c0 = j * MICRO
                    P = pnpool.tile([128, 1024], F32, tag="psn")
                    XT = TTs[0:101, c0:c0 + MICRO]
                    nc.tensor.matmul(P[:, 0:420], XT,
                                     wb[0:101, WB_TNA:WB_TNA + 420],
                                     start=True, stop=False)
                    nc.tensor.matmul(P[:, 512:932], XT,
                                     wb[0:101, WB_TNB:WB_TNB + 420],
                                     start=True, stop=False)
                    XB = Bt[:, c0:c0 + MICRO]
                    nc.tensor.matmul(P[:, 0:420], XB,
                                     wb[:, WB_UHA:WB_UHA + 420],
                                     start=False, stop=False)
                    nc.tensor.matmul(P[:, 512:932], XB,
                                     wb[:, WB_UHB:WB_UHB + 420],
                                     start=False, stop=False)
                    XK = K8[:, 0:2, c0:c0 + MICRO]
                    nc.tensor.matmul(P[:, 0:420], XK, w8r(W8_KA, 420, 432),
                                     start=False, stop=True, perf_mode=DR)
                    nc.tensor.matmul(P[:, 512:932], XK, w8r(W8_KB, 420, 432),
                                     start=False, stop=True, perf_mode=DR)
                    pr = P[:].rearrange("p (a b) -> p a b", a=2, b=512)
                    sr = sgn[:, j, :].rearrange("p (a b) -> p a b", a=2,
                                                b=420)
                    nc.scalar.activation(sr, pr[:, :, 0:420], SIGF,
                                         scale=1.0 / WS)
                s_i2 = sgn[:, :, 0:168]
                s_o2 = sgn[:, :, 168:336]
                s_fl2 = sgn[:, :, 336:504]
                s_fd2 = sgn[:, :, 504:672]
                s_u2 = sgn[:, :, 672:840]
                tu2 = gpool.tile([128, GRP, 168], BF, tag="tu2")
                nc.gpsimd.tensor_scalar(tu2[:], s_u2, 2.0, -1.0, MULT, ADD)
                m3 = gpool.tile([128, GRP, 168], BF, tag="tmpA")
                nc.vector.tensor_mul(m3[:], s_i2, tu2[:])
                m4 = gpool.tile([128, GRP, 168], BF, tag="tmpB")
                nc.vector.tensor_mul(m4[:], s_fd2, CQt[:, :, 168:336])
                m5 = gpool.tile([128, GRP, 168], BF, tag="tmpC")
                nc.vector.tensor_mul(m5[:], s_fl2, c1t[:])
                a1 = gpool.tile([128, GRP, 168], BF, tag="tmpA")
                nc.vector.tensor_add(a1[:], m3[:], m4[:])
                nc.vector.tensor_add(OUTt[:, :, 168:336], a1[:], m5[:])
                tc2 = gpool.tile([128, GRP, 168], BF, tag="tc2")
                nc.scalar.activation(tc2[:], OUTt[:, :, 168:336], TANHF)
                nc.vector.tensor_mul(OUTt[:, :, 0:168], s_o2, tc2[:])
                return OUTt

            def store_group(g, OUTt):
                cs = g * GNODES
                nc.gpsimd.dma_start(
                    out_d[cs:cs + GNODES, :].rearrange("(m p) f -> p m f",
                                                       p=128),
                    OUTt[:])

            # ---- startup: chase-load group 0 per-micro so leaf(0) can
            # start within a few us; spread group 0/1 side tensors on the
            # scalar ring (idle at startup).  No HAM warmup matmuls: real
            # leaf matmuls start almost immediately.
            def load_group0():
                EK = epool.tile([128, 4, GNODES], FP8, tag="EK")
                K8 = npool.tile([128, 2, GNODES], FP8, tag="K8")
                TTs = npool.tile([128, GNODES], BF, tag="TT")
                for j in range(GRP):
                    c0 = j * MICRO
                    nc.sync.dma_start(
                        EK[:, :, c0:c0 + MICRO],
                        x8_d[:, c0:c0 + MICRO].rearrange(
                            "(c p) n -> p c n", p=128))
                    nc.scalar.dma_start(TTs[0:101, c0:c0 + MICRO],
                                        tt1T_d[:, c0:c0 + MICRO])
                nc.scalar.dma_start(
                    K8[:], k8_d[:, 0:GNODES].rearrange(
                        "(c p) n -> p c n", p=128))
                Bt = npool.tile([128, GNODES], BF, tag="B")
                sstate[0] = dict(EK=EK, K8=K8, TT=TTs, B=Bt)

            load_group0()
            if ngrp > 1:
                load_group_main(1, split=True)
            for g in range(ngrp + NL):
                gl = g if g < ngrp else None
                gn = g - NL if g >= NL else None
                if g == 0:
                    load_group_cq(0)
                if gl is not None:
                    leaf_block(gl)
                if gn is not None:
                    store_group(gn, node_block(gn))
                if gl is not None and gl + 2 < ngrp:
                    load_group_main(gl + 2)
                if gl is not None and gl + 1 < ngrp:
                    load_group_cq(gl + 1)
                if gl is not None:
                    chain_block(gl)
                    cast_fin(gl)

    nc.compile()
    return nc


def _q8(a):
    return np.clip(np.asarray(a, np.float32), -240, 240).astype(F8)


def _prep_core(inputs, c, npad=NPAD, nper=NPER):
    """Build the per-core (sharded, transposed, fp8/bf16) input arrays."""
    sl = slice(c * nper, (c + 1) * nper)
    e = inputs["e"][sl]
    h_prev = inputs["h_prev"][sl]
    tag = inputs["tag"][sl]
    tagp = inputs["tag_parent"][sl]
    k = inputs["k"][sl]
    c_prev = inputs["c_prev"][sl]
    q = inputs["q"][sl]
    n = e.shape[0]

    x8 = np.zeros((512, npad), F8)
    x8[0:300, :n] = _q8(e.T)
    x8[300:468, :n] = _q8(h_prev.T)
    k8 = np.zeros((256, npad), F8)
    k8[0:128, :n] = _q8(k[:, 0:128].T)
    # plane 1: partitions 0:40 are the device-written h1 tail slot;
    # 40:80 carry k tail rows; rest zero.
    k8[168:208, :n] = _q8(k[:, 128:168].T)
    tt1T = np.zeros((101, npad), BF16)
    tt1T[0:50, :n] = tag.T
    tt1T[50:100, :n] = tagp.T
    tt1T[100, :n] = 1.0
    cq = np.zeros((npad, 336), BF16)
    cq[:n, 0:168] = c_prev
    cq[:n, 168:336] = q
    return dict(x8=x8, k8=k8, tt1T=tt1T, cq=cq)


def _prep_weights(inputs):
    cat = np.concatenate
    # leaf fused weights [569, 672]: rows [e(300); h_prev(168); tags(101)],
    # gate cols [i,o,fl,u]; u columns pre-scaled x2 (tanh via sigmoid),
    # everything x WS for fp8 range (sigmoid applies 1/WS).
    w_l = cat([inputs["We_l"], inputs["Uh_l"],
               inputs["Wt_l"], inputs["Wtp_l"],
               inputs["b_l"][None, :]], 0).astype(np.float64) * WS
    w_l[:, 504:672] *= 2.0
    # node fused [438, 840]: rows [tags(101); h1(168); k(168)],
    # gate cols [i,o,fl,fd,u] (u cols x2); Uk covers gates 0:672 only.
    w_n = np.zeros((437, 840), np.float64)
    w_n[0:50] = inputs["Wt_n"]
    w_n[50:100] = inputs["Wtp_n"]
    w_n[100] = inputs["b_n"]
    w_n[101:269] = inputs["Uh_n"]
    w_n[269:437, 0:672] = inputs["Uk_n"]
    w_n *= WS
    w_n[:, 672:840] *= 2.0

    w8 = np.zeros((128, W8_COLS), F8)

    def put8(base, stride, n, pl0, pl1):
        # pl0/pl1: [rows<=128, n] weight blocks for the two DR planes
        for i, pl in enumerate((pl0, pl1)):
            w8[0:pl.shape[0], base + i * stride:base + i * stride + n] = \
                _q8(pl)

    # leaf pair 1: planes e[0:128], e[128:256]
    put8(W8_L1A, 336, 336, w_l[0:128, 0:336], w_l[128:256, 0:336])
    put8(W8_L1B, 336, 336, w_l[0:128, 336:672], w_l[128:256, 336:672])
    # leaf pair 2: planes [e256:300;h0:84], [h84:168; pad44]
    put8(W8_L2A, 336, 336, w_l[256:384, 0:336], w_l[384:468, 0:336])
    put8(W8_L2B, 336, 336, w_l[256:384, 336:672], w_l[384:468, 336:672])
    # node k pair: plane0 = k rows 0:128 (Uk), plane1 = [h1 tail rows
    # 128:168 (Uh) at 0:40; k tail rows at 40:80]
    kp0 = w_n[269:397]
    kp1 = np.zeros((80, 840))
    kp1[0:40] = w_n[229:269]   # Uh rows 128:168
    kp1[40:80] = w_n[397:437]  # Uk rows 128:168
    put8(W8_KA, 432, 420, kp0[:, 0:420], kp1[:, 0:420])
    put8(W8_KB, 432, 420, kp0[:, 420:840], kp1[:, 420:840])

    wbp = np.zeros((128, WB_COLS), BF16)
    tl = w_l[468:569]          # leaf tags rows [101, 672]
    wbp[0:101, WB_TLA:WB_TLA + 336] = tl[:, 0:336].astype(BF16)
    wbp[0:101, WB_TLB:WB_TLB + 336] = tl[:, 336:672].astype(BF16)
    tn = w_n[0:101]            # node tags rows [101, 840]
    wbp[0:101, WB_TNA:WB_TNA + 420] = tn[:, 0:420].astype(BF16)
    wbp[0:101, WB_TNB:WB_TNB + 420] = tn[:, 420:840].astype(BF16)
    uh = w_n[101:229]          # Uh rows 0:128 [128, 840]
    wbp[:, WB_UHA:WB_UHA + 420] = uh[:, 0:420].astype(BF16)
    wbp[:, WB_UHB:WB_UHB + 420] = uh[:, 420:840].astype(BF16)
    return dict(w8=w8, wb=wbp)


def kernel(**inputs):
    global _compiled, LAST_RESULT
    inputs = {k: np.asarray(v) for k, v in inputs.items()}
    if _compiled is None:
        _compiled = _build()
    weights = _prep_weights(inputs)
    in_maps = []
    for c in range(N_CORES):
        m = _prep_core(inputs, c)
        m.update(weights)
        in_maps.append(m)
    res = run_bass_kernel_spmd(_compiled, in_maps,
                               core_ids=list(range(N_CORES)))
    LAST_RESULT = res
    outs = [res.results[c]["out"][:NPER].astype(np.float32)
            for c in range(N_CORES)]
    return np.concatenate(outs, 0)

